# revision 24
# baseline (speedup 1.0000x reference)
"""CRF loss (forward-algorithm logsumexp recurrence) on 8 NeuronCores.

Strategy: data-parallel over batch (B=128 -> 16 per core). The forward
recurrence runs in the exp domain: with P_t = exp(state_t - offs_t),
    P_t = (P_{t-1} @ E) * F_t,   E = exp(transitions), F_t = exp(f_t - c_t)
so each step is two 128-contraction matmul pairs (state kept transposed
as [k, b] so the constant E tiles are the stationary operand) plus one
elementwise multiply per PSUM bank. Per-step normalizers c_t
(host-computed from the feature frames) keep P in a tight dynamic
range, so fp16 operands are safe. A ones-vector matmul accumulates
s_t[b] = sum_k P_t[k, b] once per 16-step window into an on-chip log;
the host picks s at t = len[b]-1, takes the log, re-adds the
accumulated normalizers, and subtracts the gold-path score (a cheap
O(B*T) gather done on host).

The per-step cycle is latency-bound: MM-pair drain (~170ns) + sem hop
(~90ns) + DVE multiply (~175ns) + sem hop (~54ns) + issue (~30ns)
~= 519ns, so the remaining wins are overhead: the s-emission matmul
runs once per 16 steps (N=512, mostly hidden in PE idle), the weight /
first-chunk DMAs are spread across idle engine queues, PE warm-up
matmuls run during the initial DMA wait (pays the HAM/p-state ramp
early), and step 1 reads the F chunk directly as its moving operand so
the state0 copy is off the critical path.
"""

import numpy as np

B, T, K = 128, 256, 256
N_CORES = 8
BL = B // N_CORES  # batch per core
KT = K // 128      # k tiles (contraction/output splits)
CHUNK = 32         # timesteps of F per DMA chunk

_cache = {}


def _build_nc(t_eff):
    from contextlib import ExitStack

    import concourse.bacc as bacc
    import concourse.tile as tile
    from concourse import mybir

    nc = bacc.Bacc("TRN2", target_bir_lowering=False, debug=False,
                   enable_asserts=False, num_devices=N_CORES)
    f16 = mybir.dt.float16
    f32 = mybir.dt.float32

    e_in = nc.dram_tensor("e_in", [128, KT * KT * 128], f16,
                          kind="ExternalInput").ap()
    # F[k, t*2*BL + j*BL + b] = exp(f[t, b, 128j + k] - c_t)
    f_in = nc.dram_tensor("f_in", [128, T * 2 * BL], f16,
                          kind="ExternalInput").ap()
    s_out = nc.dram_tensor("s_out", [1, T * 2 * BL], f32,
                           kind="ExternalOutput").ap()

    WIN = 8          # P' slots per state buffer; s-sum matmul per window
    with tile.TileContext(nc) as tc, ExitStack() as ctx:
        consts = ctx.enter_context(tc.tile_pool(name="consts", bufs=1))
        fpool = ctx.enter_context(tc.tile_pool(name="fpool", bufs=3))
        state = ctx.enter_context(tc.tile_pool(name="state", bufs=2))
        psum = ctx.enter_context(tc.tile_pool(name="psum", bufs=2,
                                              space="PSUM"))
        psum_s = ctx.enter_context(tc.tile_pool(name="psum_s", bufs=2,
                                                space="PSUM"))
        psum_w = ctx.enter_context(tc.tile_pool(name="psum_w", bufs=1,
                                                space="PSUM"))

        # variable chunk sizes: a tiny first chunk so step 0 isn't gated
        # on a large DMA, full-size chunks after, remainder last
        chunk_lens = [4, CHUNK - 4] + [CHUNK] * ((t_eff - CHUNK) // CHUNK)
        rem = t_eff - sum(chunk_lens)
        if rem:
            chunk_lens.append(rem)
        chunk_t0 = np.cumsum([0] + chunk_lens).tolist()
        n_chunks = len(chunk_lens)
        step_chunk = {}
        for ci, (t0c, ln) in enumerate(zip(chunk_t0, chunk_lens)):
            for tt_ in range(t0c, t0c + ln):
                step_chunk[tt_] = (ci, tt_ - t0c)
        fch = [None] * n_chunks

        def load_chunk(c, split=1, eng=None):
            ln = chunk_lens[c]
            ft = fpool.tile([128, CHUNK * 2 * BL], f16, tag="f", name="fch")
            w = ln * 2 * BL
            base = chunk_t0[c] * 2 * BL
            for s in range(split):
                lo, hi = s * w // split, (s + 1) * w // split
                (eng or nc.sync).dma_start(ft[:, lo:hi],
                                           f_in[:, base + lo:base + hi])
            fch[c] = ft

        # first data: chunk 0 heads the sync ring (it gates step 1's
        # moving operand); the E tiles (packed host-side in consumption
        # order e00,e10,e01,e11) load per-tile, split 2+2 across the
        # sync + gpsimd rings so the laggard tile lands soonest
        e_all = consts.tile([128, KT * KT * 128], f16, tag="eall",
                            name="eall")
        q = 128
        load_chunk(0, split=1)
        nc.gpsimd.dma_start(e_all[:, 0:q], e_in[:, 0:q])          # e00
        nc.sync.dma_start(e_all[:, q:2 * q], e_in[:, q:2 * q])    # e10
        nc.gpsimd.dma_start(e_all[:, 2 * q:3 * q],
                            e_in[:, 2 * q:3 * q])                 # e01
        nc.sync.dma_start(e_all[:, 3 * q:4 * q],
                          e_in[:, 3 * q:4 * q])                   # e11
        # slot s = (k-tile, out-tile) in step consumption order:
        # (0,0), (1,0), (0,1), (1,1); e_t[k][i] -> slot i*KT + k
        e_t = [[e_all[:, (i * KT + k) * 128:(i * KT + k + 1) * 128]
                for i in range(KT)] for k in range(KT)]
        ones = consts.tile([128, 1], f16, tag="ones", name="ones")
        nc.vector.memset(ones[:], 1.0)

        # s log: one row, all timesteps x (j, b) partials
        s_buf = consts.tile([1, T * 2 * BL], f32, tag="sbuf", name="sbuf")

        load_chunk(1, split=2)

        # PE warm-up while the first DMAs land: burn the HAM/p-state
        # ramp on scratch matmuls instead of the first real steps
        scratch = consts.tile([128, 256], f16, tag="scr", name="scr")
        nc.vector.memset(scratch[:], 1.0)
        psw = psum_w.tile([128, 256], f32, tag="psw", name="psw")
        for _ in range(2):
            nc.tensor.matmul(psw[:], scratch[:, 0:128], scratch[:],
                             start=True, stop=True)

        def emit_s(w, pb, ncols):
            pss = psum_s.tile([1, WIN * 2 * BL], f32, tag="pss", name="pss")
            nc.tensor.matmul(pss[:, 0:ncols], ones[:], pb[:, 0:ncols],
                             start=True, stop=True)
            lo = w * WIN * 2 * BL
            nc.scalar.copy(s_buf[:, lo:lo + ncols], pss[:, 0:ncols])
            nc.sync.dma_start(s_out[:, lo:lo + ncols],
                              s_buf[:, lo:lo + ncols])

        # P' tiles: WIN step-slots of 32 cols in one [128, WIN*32] buffer,
        # double-buffered by window. Column layout per slot: j*BL + b.
        pbuf_prev = None
        pending_s = None
        pbuf = state.tile([128, WIN * 2 * BL], f16, tag="pb", name="pb")
        for t in range(t_eff):
            c, r = step_chunk[t]
            if r == 0 and c + 2 < n_chunks:
                load_chunk(c + 2)
            w, slot = divmod(t, WIN)
            fcol = fch[c][:, r * 2 * BL:(r + 1) * 2 * BL]   # [128, 32]
            p_new = pbuf[:, slot * 2 * BL:(slot + 1) * 2 * BL]
            if t == 0:
                # state0 = F0; the copy only feeds the s-emission --
                # step 1 reads the chunk tile directly
                nc.vector.tensor_copy(p_new, fcol)
            else:
                pt = t - 1
                if pt == 0:
                    c0, r0 = step_chunk[0]
                    p_prev = fch[c0][:, r0 * 2 * BL:(r0 + 1) * 2 * BL]
                else:
                    pslot = pt % WIN
                    src = pbuf_prev if pslot == WIN - 1 else pbuf
                    p_prev = src[:, pslot * 2 * BL:(pslot + 1) * 2 * BL]
                # two PSUM banks so the DVE can multiply half 0 while the
                # PE is still writing half 1 (same-bank PE-W/DVE-R would
                # serialize)
                ps0 = psum.tile([128, BL], f32, tag="ps0", name="ps0")
                ps1 = psum.tile([128, BL], f32, tag="ps1", name="ps1")
                nc.tensor.matmul(ps0[:], e_t[0][0][:],
                                 p_prev[:, 0:BL], start=True, stop=False)
                nc.tensor.matmul(ps0[:], e_t[1][0][:],
                                 p_prev[:, BL:2 * BL], start=False, stop=True)
                nc.tensor.matmul(ps1[:], e_t[0][1][:],
                                 p_prev[:, 0:BL], start=True, stop=False)
                nc.tensor.matmul(ps1[:], e_t[1][1][:],
                                 p_prev[:, BL:2 * BL], start=False, stop=True)
                nc.vector.tensor_mul(p_new[:, 0:BL], ps0[:], fcol[:, 0:BL])
                nc.vector.tensor_mul(p_new[:, BL:2 * BL], ps1[:],
                                     fcol[:, BL:2 * BL])
            if pending_s is not None and slot == 1:
                emit_s(*pending_s)
                pending_s = None
            if slot == WIN - 1 or t == t_eff - 1:
                # s partials for the whole window in one matmul; deferred
                # to early next window so it fills a PE idle gap instead
                # of blocking the next step's matmuls in the PE FIFO
                pending_s = (w, pbuf, (slot + 1) * 2 * BL)
                pbuf_prev = pbuf
                pbuf = state.tile([128, WIN * 2 * BL], f16, tag="pb",
                                  name="pb")

        if pending_s is not None:
            emit_s(*pending_s)

    nc.compile()
    return nc


def _prepare(feats, transitions, feats_len):
    f = np.ascontiguousarray(feats.transpose(1, 0, 2)).astype(np.float32)
    # per-step normalizer: mean over batch of logsumexp_k of the frame
    m = f.max(axis=2)
    lse = np.log(np.exp(f - m[:, :, None]).sum(axis=2,
                                               dtype=np.float32)) + m
    c = lse.mean(axis=1).astype(np.float32)             # [T]
    offs = np.cumsum(c.astype(np.float64))              # [T]

    E = np.exp(transitions.astype(np.float32))
    # slot i*KT + k holds E[k-tile, i-tile] (step consumption order)
    e_packed = np.empty((128, KT * KT * 128), np.float16)
    for i in range(KT):
        for k in range(KT):
            e_packed[:, (i * KT + k) * 128:(i * KT + k + 1) * 128] = \
                E[128 * k:128 * (k + 1), 128 * i:128 * (i + 1)]

    # F[t, b, k] = exp(f[t, b, k] - c_t) -> [128, T*2*BL] per core
    Fx = np.exp(f - c[:, None, None]).astype(np.float32)  # [T, B, K]
    f_maps = []
    for core in range(N_CORES):
        sl = Fx[:, core * BL:(core + 1) * BL, :]          # [T, BL, K]
        blk = sl.reshape(T, BL, KT, 128).transpose(3, 0, 2, 1)
        f_maps.append({"f_in": np.ascontiguousarray(
            blk.reshape(128, T * KT * BL)).astype(np.float16)})
    return e_packed, f_maps, offs, c


def _gold_score(feats, transitions, tags, feats_len):
    f = feats.transpose(1, 0, 2).astype(np.float32)       # [T, B, K]
    tg = tags.T.astype(np.int64)                          # [T, B]
    mask = (np.arange(T)[:, None] < feats_len[None, :])
    maskf = mask.astype(np.float32)
    emit = np.take_along_axis(f, tg[:, :, None], axis=2)[:, :, 0] * maskf
    u = emit.sum(axis=0, dtype=np.float32)
    t_mask = maskf[:-1] * maskf[1:]
    t_score = transitions.astype(np.float32)[tg[:-1], tg[1:]] * t_mask
    return (u + t_score.sum(axis=0, dtype=np.float32)).astype(np.float32)


def kernel(feats, transitions, tags, feats_len, _results_hook=None,
           _trace=False):
    from concourse.bass_utils import run_bass_kernel_spmd

    feats = np.asarray(feats, dtype=np.float32)
    transitions = np.asarray(transitions, dtype=np.float32)
    tags_np = np.asarray(tags)
    feats_len_np = np.asarray(feats_len).astype(np.int64)

    # compile only max(len) steps; floor keeps the chunk schedule valid
    t_eff = min(T, max(int(feats_len_np.max()), 2 * CHUNK))
    if ("nc", t_eff) not in _cache:
        _cache[("nc", t_eff)] = _build_nc(t_eff)
    nc = _cache[("nc", t_eff)]

    e_packed, f_maps, offs, _c = _prepare(feats, transitions, feats_len_np)
    in_maps = [{"e_in": e_packed, **f_maps[core]} for core in range(N_CORES)]

    res = run_bass_kernel_spmd(nc, in_maps, core_ids=list(range(N_CORES)),
                               trace=_trace)
    if _results_hook is not None:
        _results_hook(res)

    u = _gold_score(feats, transitions, tags_np, feats_len_np)
    loss = np.empty(B, np.float32)
    idx = feats_len_np - 1                                 # [B] capture step
    for core in range(N_CORES):
        s = res.results[core]["s_out"].reshape(T, KT, BL).astype(
            np.float64).sum(axis=1)                        # [T, BL]
        bl = np.arange(BL)
        bg = core * BL + bl
        sv = s[idx[bg], bl]
        loss[bg] = (np.log(sv) + offs[idx[bg]]).astype(np.float32) - u[bg]
    return loss


# revision 29
# speedup vs baseline: 1.1899x; 1.1899x over previous
"""CRF loss (forward-algorithm logsumexp recurrence) on 8 NeuronCores.

Strategy: data-parallel over batch (B=128 -> 16 per core). The forward
recurrence runs in the exp domain: with P_t = exp(state_t - offs_t),
    P_t = (P_{t-1} @ E) * F_t,   E = exp(transitions), F_t = exp(f_t - c_t)
so each step is two 128-contraction matmul pairs (state kept transposed
as [k, b] so the constant E tiles are the stationary operand) plus one
elementwise multiply per PSUM bank. Per-step normalizers c_t
(host-computed from the feature frames) keep P in a tight dynamic
range, so fp16 operands are safe. A ones-vector matmul accumulates
s_t[b] = sum_k P_t[k, b] once per 16-step window into an on-chip log;
the host picks s at t = len[b]-1, takes the log, re-adds the
accumulated normalizers, and subtracts the gold-path score (a cheap
O(B*T) gather done on host).

The per-step cycle is latency-bound: MM-pair drain (~170ns) + sem hop
(~90ns) + DVE multiply (~175ns) + sem hop (~54ns) + issue (~30ns)
~= 519ns, so the remaining wins are overhead: the s-emission matmul
runs once per 16 steps (N=512, mostly hidden in PE idle), the weight /
first-chunk DMAs are spread across idle engine queues, PE warm-up
matmuls run during the initial DMA wait (pays the HAM/p-state ramp
early), and step 1 reads the F chunk directly as its moving operand so
the state0 copy is off the critical path.
"""

import numpy as np

B, T, K = 128, 256, 256
N_CORES = 8
BL = B // N_CORES  # batch per core
KT = K // 128      # k tiles (contraction/output splits)
CHUNK = 32         # timesteps of F per DMA chunk

_cache = {}


def _build_nc(t_eff):
    from contextlib import ExitStack

    import concourse.bacc as bacc
    import concourse.tile as tile
    from concourse import mybir

    nc = bacc.Bacc("TRN2", target_bir_lowering=False, debug=False,
                   enable_asserts=False, num_devices=N_CORES)
    f16 = mybir.dt.float16
    f32 = mybir.dt.float32

    e_in = nc.dram_tensor("e_in", [128, KT * KT * 128], f16,
                          kind="ExternalInput").ap()
    # F[k, t*2*BL + j*BL + b] = exp(f[t, b, 128j + k] - c_t)
    f_in = nc.dram_tensor("f_in", [128, T * 2 * BL], f16,
                          kind="ExternalInput").ap()
    s_out = nc.dram_tensor("s_out", [1, T * 2 * BL], f32,
                           kind="ExternalOutput").ap()

    WIN = 8          # P' slots per state buffer; s-sum matmul per window
    with tile.TileContext(nc) as tc, ExitStack() as ctx:
        consts = ctx.enter_context(tc.tile_pool(name="consts", bufs=1))
        fpool = ctx.enter_context(tc.tile_pool(name="fpool", bufs=3))
        state = ctx.enter_context(tc.tile_pool(name="state", bufs=2))
        psum = ctx.enter_context(tc.tile_pool(name="psum", bufs=2,
                                              space="PSUM"))
        psum_s = ctx.enter_context(tc.tile_pool(name="psum_s", bufs=2,
                                                space="PSUM"))
        psum_w = ctx.enter_context(tc.tile_pool(name="psum_w", bufs=1,
                                                space="PSUM"))

        # variable chunk sizes: a tiny first chunk so step 0 isn't gated
        # on a large DMA, full-size chunks after, remainder last
        chunk_lens = [4, CHUNK - 4] + [CHUNK] * ((t_eff - CHUNK) // CHUNK)
        rem = t_eff - sum(chunk_lens)
        if rem:
            chunk_lens.append(rem)
        chunk_t0 = np.cumsum([0] + chunk_lens).tolist()
        n_chunks = len(chunk_lens)
        step_chunk = {}
        for ci, (t0c, ln) in enumerate(zip(chunk_t0, chunk_lens)):
            for tt_ in range(t0c, t0c + ln):
                step_chunk[tt_] = (ci, tt_ - t0c)
        fch = [None] * n_chunks

        def load_chunk(c, split=1, eng=None):
            ln = chunk_lens[c]
            ft = fpool.tile([128, CHUNK * 2 * BL], f16, tag="f", name="fch")
            w = ln * 2 * BL
            base = chunk_t0[c] * 2 * BL
            for s in range(split):
                lo, hi = s * w // split, (s + 1) * w // split
                (eng or nc.sync).dma_start(ft[:, lo:hi],
                                           f_in[:, base + lo:base + hi])
            fch[c] = ft

        # first data: chunk 0 heads the sync ring (it gates step 1's
        # moving operand); the E tiles (packed host-side in consumption
        # order e00,e10,e01,e11) load per-tile, split 2+2 across the
        # sync + gpsimd rings so the laggard tile lands soonest
        e_all = consts.tile([128, KT * KT * 128], f16, tag="eall",
                            name="eall")
        q = 128
        load_chunk(0, split=2)
        nc.gpsimd.dma_start(e_all[:, 0:q], e_in[:, 0:q])          # e00
        nc.sync.dma_start(e_all[:, q:2 * q], e_in[:, q:2 * q])    # e10
        nc.gpsimd.dma_start(e_all[:, 2 * q:3 * q],
                            e_in[:, 2 * q:3 * q])                 # e01
        nc.sync.dma_start(e_all[:, 3 * q:4 * q],
                          e_in[:, 3 * q:4 * q])                   # e11
        # slot s = (k-tile, out-tile) in step consumption order:
        # (0,0), (1,0), (0,1), (1,1); e_t[k][i] -> slot i*KT + k
        e_t = [[e_all[:, (i * KT + k) * 128:(i * KT + k + 1) * 128]
                for i in range(KT)] for k in range(KT)]
        ones = consts.tile([128, 1], f16, tag="ones", name="ones")
        nc.vector.memset(ones[:], 1.0)

        # s log: one row, all timesteps x (j, b) partials
        s_buf = consts.tile([1, T * 2 * BL], f32, tag="sbuf", name="sbuf")

        load_chunk(1, split=2)

        # PE warm-up while the first DMAs land: burn the HAM/p-state
        # ramp on scratch matmuls instead of the first real steps
        scratch = consts.tile([128, 256], f16, tag="scr", name="scr")
        nc.vector.memset(scratch[:], 1.0)
        psw = psum_w.tile([128, 256], f32, tag="psw", name="psw")
        for _ in range(2):
            nc.tensor.matmul(psw[:], scratch[:, 0:128], scratch[:],
                             start=True, stop=True)

        def emit_s(w, pb, ncols, c0=0, eng=None):
            pss = psum_s.tile([1, WIN * 2 * BL], f32, tag="pss", name="pss")
            nc.tensor.matmul(pss[:, 0:ncols - c0], ones[:], pb[:, c0:ncols],
                             start=True, stop=True)
            lo = w * WIN * 2 * BL + c0
            cp = nc.vector.tensor_copy if eng is nc.vector else nc.scalar.copy
            cp(s_buf[:, lo:lo + ncols - c0], pss[:, 0:ncols - c0])
            nc.sync.dma_start(s_out[:, lo:lo + ncols - c0],
                              s_buf[:, lo:lo + ncols - c0])

        # P' tiles: WIN step-slots of 32 cols in one [128, WIN*32] buffer,
        # double-buffered by window. Column layout per slot: j*BL + b.
        pbuf_prev = None
        pending_s = None
        pbuf = state.tile([128, WIN * 2 * BL], f16, tag="pb", name="pb")
        for t in range(t_eff):
            c, r = step_chunk[t]
            if r == 0 and c + 2 < n_chunks:
                load_chunk(c + 2)
            w, slot = divmod(t, WIN)
            fcol = fch[c][:, r * 2 * BL:(r + 1) * 2 * BL]   # [128, 32]
            p_new = pbuf[:, slot * 2 * BL:(slot + 1) * 2 * BL]
            if t == 0:
                # state0 = F0; the copy only feeds the s-emission --
                # step 1 reads the chunk tile directly
                nc.vector.tensor_copy(p_new, fcol)
            else:
                pt = t - 1
                if pt == 0:
                    c0, r0 = step_chunk[0]
                    p_prev = fch[c0][:, r0 * 2 * BL:(r0 + 1) * 2 * BL]
                else:
                    pslot = pt % WIN
                    src = pbuf_prev if pslot == WIN - 1 else pbuf
                    p_prev = src[:, pslot * 2 * BL:(pslot + 1) * 2 * BL]
                # two PSUM banks so the DVE can multiply half 0 while the
                # PE is still writing half 1 (same-bank PE-W/DVE-R would
                # serialize)
                ps0 = psum.tile([128, BL], f32, tag="ps0", name="ps0")
                ps1 = psum.tile([128, BL], f32, tag="ps1", name="ps1")
                nc.tensor.matmul(ps0[:], e_t[0][0][:],
                                 p_prev[:, 0:BL], start=True, stop=False)
                nc.tensor.matmul(ps0[:], e_t[1][0][:],
                                 p_prev[:, BL:2 * BL], start=False, stop=True)
                nc.tensor.matmul(ps1[:], e_t[0][1][:],
                                 p_prev[:, 0:BL], start=True, stop=False)
                nc.tensor.matmul(ps1[:], e_t[1][1][:],
                                 p_prev[:, BL:2 * BL], start=False, stop=True)
                nc.vector.tensor_mul(p_new[:, 0:BL], ps0[:], fcol[:, 0:BL])
                nc.vector.tensor_mul(p_new[:, BL:2 * BL], ps1[:],
                                     fcol[:, BL:2 * BL])
            if pending_s is not None and slot == 1:
                emit_s(*pending_s)
                pending_s = None
            w_last = (t_eff - 1) // WIN
            if w == w_last and slot == WIN - 3:
                # final window: emit the completed slots now so the
                # post-loop emission (on the serial tail) is minimal
                emit_s(w, pbuf, (WIN - 4) * 2 * BL)
            if t == t_eff - 1:
                if pending_s is not None:
                    emit_s(*pending_s)
                    pending_s = None
                c0 = (WIN - 4) * 2 * BL if slot >= WIN - 3 else 0
                emit_s(w, pbuf, (slot + 1) * 2 * BL, c0=c0, eng=nc.vector)
            elif slot == WIN - 1:
                # s partials for the whole window in one matmul; deferred
                # to early next window so it fills a PE idle gap instead
                # of blocking the next step's matmuls in the PE FIFO
                pending_s = (w, pbuf, (slot + 1) * 2 * BL)
                pbuf_prev = pbuf
                pbuf = state.tile([128, WIN * 2 * BL], f16, tag="pb",
                                  name="pb")

    nc.compile()
    return nc


def _prepare(feats, transitions, feats_len):
    f = np.ascontiguousarray(feats.transpose(1, 0, 2)).astype(np.float32)
    # per-step normalizer: mean over batch of logsumexp_k of the frame
    m = f.max(axis=2)
    lse = np.log(np.exp(f - m[:, :, None]).sum(axis=2,
                                               dtype=np.float32)) + m
    c = lse.mean(axis=1).astype(np.float32)             # [T]
    offs = np.cumsum(c.astype(np.float64))              # [T]

    E = np.exp(transitions.astype(np.float32))
    # slot i*KT + k holds E[k-tile, i-tile] (step consumption order)
    e_packed = np.empty((128, KT * KT * 128), np.float16)
    for i in range(KT):
        for k in range(KT):
            e_packed[:, (i * KT + k) * 128:(i * KT + k + 1) * 128] = \
                E[128 * k:128 * (k + 1), 128 * i:128 * (i + 1)]

    # F[t, b, k] = exp(f[t, b, k] - c_t) -> [128, T*2*BL] per core
    Fx = np.exp(f - c[:, None, None]).astype(np.float32)  # [T, B, K]
    f_maps = []
    for core in range(N_CORES):
        sl = Fx[:, core * BL:(core + 1) * BL, :]          # [T, BL, K]
        blk = sl.reshape(T, BL, KT, 128).transpose(3, 0, 2, 1)
        f_maps.append({"f_in": np.ascontiguousarray(
            blk.reshape(128, T * KT * BL)).astype(np.float16)})
    return e_packed, f_maps, offs, c


def _gold_score(feats, transitions, tags, feats_len):
    f = feats.transpose(1, 0, 2).astype(np.float32)       # [T, B, K]
    tg = tags.T.astype(np.int64)                          # [T, B]
    mask = (np.arange(T)[:, None] < feats_len[None, :])
    maskf = mask.astype(np.float32)
    emit = np.take_along_axis(f, tg[:, :, None], axis=2)[:, :, 0] * maskf
    u = emit.sum(axis=0, dtype=np.float32)
    t_mask = maskf[:-1] * maskf[1:]
    t_score = transitions.astype(np.float32)[tg[:-1], tg[1:]] * t_mask
    return (u + t_score.sum(axis=0, dtype=np.float32)).astype(np.float32)


def kernel(feats, transitions, tags, feats_len, _results_hook=None,
           _trace=False):
    from concourse.bass_utils import run_bass_kernel_spmd

    feats = np.asarray(feats, dtype=np.float32)
    transitions = np.asarray(transitions, dtype=np.float32)
    tags_np = np.asarray(tags)
    feats_len_np = np.asarray(feats_len).astype(np.int64)

    # compile only max(len) steps; floor keeps the chunk schedule valid
    t_eff = min(T, max(int(feats_len_np.max()), 2 * CHUNK))
    if ("nc", t_eff) not in _cache:
        _cache[("nc", t_eff)] = _build_nc(t_eff)
    nc = _cache[("nc", t_eff)]

    e_packed, f_maps, offs, _c = _prepare(feats, transitions, feats_len_np)
    in_maps = [{"e_in": e_packed, **f_maps[core]} for core in range(N_CORES)]

    res = run_bass_kernel_spmd(nc, in_maps, core_ids=list(range(N_CORES)),
                               trace=_trace)
    if _results_hook is not None:
        _results_hook(res)

    u = _gold_score(feats, transitions, tags_np, feats_len_np)
    loss = np.empty(B, np.float32)
    idx = feats_len_np - 1                                 # [B] capture step
    for core in range(N_CORES):
        s = res.results[core]["s_out"].reshape(T, KT, BL).astype(
            np.float64).sum(axis=1)                        # [T, BL]
        bl = np.arange(BL)
        bg = core * BL + bl
        sv = s[idx[bg], bl]
        loss[bg] = (np.log(sv) + offs[idx[bg]]).astype(np.float32) - u[bg]
    return loss


# revision 30
# speedup vs baseline: 1.2004x; 1.0089x over previous
"""CRF loss (forward-algorithm logsumexp recurrence) on 8 NeuronCores.

Strategy: data-parallel over batch (B=128 -> 16 per core). The forward
recurrence runs in the exp domain: with P_t = exp(state_t - offs_t),
    P_t = (P_{t-1} @ E) * F_t,   E = exp(transitions), F_t = exp(f_t - c_t)
so each step is two 128-contraction matmul pairs (state kept transposed
as [k, b] so the constant E tiles are the stationary operand) plus one
elementwise multiply per PSUM bank. Per-step normalizers c_t
(host-computed from the feature frames) keep P in a tight dynamic
range, so fp16 operands are safe. A ones-vector matmul accumulates
s_t[b] = sum_k P_t[k, b] once per 16-step window into an on-chip log;
the host picks s at t = len[b]-1, takes the log, re-adds the
accumulated normalizers, and subtracts the gold-path score (a cheap
O(B*T) gather done on host).

The per-step cycle is latency-bound: MM-pair drain (~170ns) + sem hop
(~90ns) + DVE multiply (~175ns) + sem hop (~54ns) + issue (~30ns)
~= 519ns, so the remaining wins are overhead: the s-emission matmul
runs once per 16 steps (N=512, mostly hidden in PE idle), the weight /
first-chunk DMAs are spread across idle engine queues, PE warm-up
matmuls run during the initial DMA wait (pays the HAM/p-state ramp
early), and step 1 reads the F chunk directly as its moving operand so
the state0 copy is off the critical path.
"""

import numpy as np

B, T, K = 128, 256, 256
N_CORES = 8
BL = B // N_CORES  # batch per core
KT = K // 128      # k tiles (contraction/output splits)
CHUNK = 32         # timesteps of F per DMA chunk

_cache = {}


def _build_nc(t_eff):
    from contextlib import ExitStack

    import concourse.bacc as bacc
    import concourse.tile as tile
    from concourse import mybir

    nc = bacc.Bacc("TRN2", target_bir_lowering=False, debug=False,
                   enable_asserts=False, num_devices=N_CORES)
    f16 = mybir.dt.float16
    f32 = mybir.dt.float32

    e_in = nc.dram_tensor("e_in", [128, KT * KT * 128], f16,
                          kind="ExternalInput").ap()
    # F[k, t*2*BL + j*BL + b] = exp(f[t, b, 128j + k] - c_t)
    f_in = nc.dram_tensor("f_in", [128, T * 2 * BL], f16,
                          kind="ExternalInput").ap()
    s_out = nc.dram_tensor("s_out", [1, T * 2 * BL], f32,
                           kind="ExternalOutput").ap()

    WIN = 8          # P' slots per state buffer; s-sum matmul per window
    with tile.TileContext(nc) as tc, ExitStack() as ctx:
        consts = ctx.enter_context(tc.tile_pool(name="consts", bufs=1))
        fpool = ctx.enter_context(tc.tile_pool(name="fpool", bufs=3))
        state = ctx.enter_context(tc.tile_pool(name="state", bufs=2))
        psum = ctx.enter_context(tc.tile_pool(name="psum", bufs=2,
                                              space="PSUM"))
        psum_s = ctx.enter_context(tc.tile_pool(name="psum_s", bufs=2,
                                                space="PSUM"))
        psum_w = ctx.enter_context(tc.tile_pool(name="psum_w", bufs=1,
                                                space="PSUM"))

        # variable chunk sizes: a tiny first chunk so step 0 isn't gated
        # on a large DMA, full-size chunks after, remainder last
        chunk_lens = [4, CHUNK - 4] + [CHUNK] * ((t_eff - CHUNK) // CHUNK)
        rem = t_eff - sum(chunk_lens)
        if rem:
            chunk_lens.append(rem)
        chunk_t0 = np.cumsum([0] + chunk_lens).tolist()
        n_chunks = len(chunk_lens)
        step_chunk = {}
        for ci, (t0c, ln) in enumerate(zip(chunk_t0, chunk_lens)):
            for tt_ in range(t0c, t0c + ln):
                step_chunk[tt_] = (ci, tt_ - t0c)
        fch = [None] * n_chunks

        def load_chunk(c, split=1, eng=None):
            ln = chunk_lens[c]
            ft = fpool.tile([128, CHUNK * 2 * BL], f16, tag="f", name="fch")
            w = ln * 2 * BL
            base = chunk_t0[c] * 2 * BL
            for s in range(split):
                lo, hi = s * w // split, (s + 1) * w // split
                (eng or nc.sync).dma_start(ft[:, lo:hi],
                                           f_in[:, base + lo:base + hi])
            fch[c] = ft

        # first data: chunk 0 heads the sync ring (it gates step 1's
        # moving operand); the E tiles (packed host-side in consumption
        # order e00,e10,e01,e11) load per-tile, split 2+2 across the
        # sync + gpsimd rings so the laggard tile lands soonest
        e_all = consts.tile([128, KT * KT * 128], f16, tag="eall",
                            name="eall")
        q = 128
        load_chunk(0, split=1)
        nc.gpsimd.dma_start(e_all[:, 0:q], e_in[:, 0:q])          # e00
        nc.sync.dma_start(e_all[:, q:2 * q], e_in[:, q:2 * q])    # e10
        nc.gpsimd.dma_start(e_all[:, 2 * q:3 * q],
                            e_in[:, 2 * q:3 * q])                 # e01
        nc.sync.dma_start(e_all[:, 3 * q:4 * q],
                          e_in[:, 3 * q:4 * q])                   # e11
        # slot s = (k-tile, out-tile) in step consumption order:
        # (0,0), (1,0), (0,1), (1,1); e_t[k][i] -> slot i*KT + k
        e_t = [[e_all[:, (i * KT + k) * 128:(i * KT + k + 1) * 128]
                for i in range(KT)] for k in range(KT)]
        ones = consts.tile([128, 1], f16, tag="ones", name="ones")
        nc.vector.memset(ones[:], 1.0)

        # s log: one row, all timesteps x (j, b) partials
        s_buf = consts.tile([1, T * 2 * BL], f32, tag="sbuf", name="sbuf")

        load_chunk(1, split=2)

        # PE warm-up while the first DMAs land: burn the HAM/p-state
        # ramp on scratch matmuls instead of the first real steps
        scratch = consts.tile([128, 256], f16, tag="scr", name="scr")
        nc.vector.memset(scratch[:], 1.0)
        psw = psum_w.tile([128, 256], f32, tag="psw", name="psw")
        for _ in range(2):
            nc.tensor.matmul(psw[:], scratch[:, 0:128], scratch[:],
                             start=True, stop=True)

        def emit_s(w, pb, ncols, c0=0, eng=None):
            pss = psum_s.tile([1, WIN * 2 * BL], f32, tag="pss", name="pss")
            nc.tensor.matmul(pss[:, 0:ncols - c0], ones[:], pb[:, c0:ncols],
                             start=True, stop=True)
            lo = w * WIN * 2 * BL + c0
            cp = nc.vector.tensor_copy if eng is nc.vector else nc.scalar.copy
            cp(s_buf[:, lo:lo + ncols - c0], pss[:, 0:ncols - c0])
            nc.sync.dma_start(s_out[:, lo:lo + ncols - c0],
                              s_buf[:, lo:lo + ncols - c0])

        # P' tiles: WIN step-slots of 32 cols in one [128, WIN*32] buffer,
        # double-buffered by window. Column layout per slot: j*BL + b.
        pbuf_prev = None
        pending_s = None
        pbuf = state.tile([128, WIN * 2 * BL], f16, tag="pb", name="pb")
        for t in range(t_eff):
            c, r = step_chunk[t]
            if r == 0 and c + 2 < n_chunks:
                load_chunk(c + 2)
            w, slot = divmod(t, WIN)
            fcol = fch[c][:, r * 2 * BL:(r + 1) * 2 * BL]   # [128, 32]
            p_new = pbuf[:, slot * 2 * BL:(slot + 1) * 2 * BL]
            if t == 0:
                # state0 = F0; the copy only feeds the s-emission --
                # step 1 reads the chunk tile directly
                nc.vector.tensor_copy(p_new, fcol)
            else:
                pt = t - 1
                if pt == 0:
                    c0, r0 = step_chunk[0]
                    p_prev = fch[c0][:, r0 * 2 * BL:(r0 + 1) * 2 * BL]
                else:
                    pslot = pt % WIN
                    src = pbuf_prev if pslot == WIN - 1 else pbuf
                    p_prev = src[:, pslot * 2 * BL:(pslot + 1) * 2 * BL]
                # two PSUM banks so the DVE can multiply half 0 while the
                # PE is still writing half 1 (same-bank PE-W/DVE-R would
                # serialize)
                ps0 = psum.tile([128, BL], f32, tag="ps0", name="ps0")
                ps1 = psum.tile([128, BL], f32, tag="ps1", name="ps1")
                nc.tensor.matmul(ps0[:], e_t[0][0][:],
                                 p_prev[:, 0:BL], start=True, stop=False)
                nc.tensor.matmul(ps0[:], e_t[1][0][:],
                                 p_prev[:, BL:2 * BL], start=False, stop=True)
                nc.tensor.matmul(ps1[:], e_t[0][1][:],
                                 p_prev[:, 0:BL], start=True, stop=False)
                nc.tensor.matmul(ps1[:], e_t[1][1][:],
                                 p_prev[:, BL:2 * BL], start=False, stop=True)
                nc.vector.tensor_mul(p_new[:, 0:BL], ps0[:], fcol[:, 0:BL])
                nc.vector.tensor_mul(p_new[:, BL:2 * BL], ps1[:],
                                     fcol[:, BL:2 * BL])
            if pending_s is not None and slot == 1:
                emit_s(*pending_s)
                pending_s = None
            w_last = (t_eff - 1) // WIN
            if w == w_last and slot == WIN - 3:
                # final window: emit the completed slots now so the
                # post-loop emission (on the serial tail) is minimal
                emit_s(w, pbuf, (WIN - 4) * 2 * BL)
            if t == t_eff - 1:
                if pending_s is not None:
                    emit_s(*pending_s)
                    pending_s = None
                c0 = (WIN - 4) * 2 * BL if slot >= WIN - 3 else 0
                emit_s(w, pbuf, (slot + 1) * 2 * BL, c0=c0, eng=nc.vector)
            elif slot == WIN - 1:
                # s partials for the whole window in one matmul; deferred
                # to early next window so it fills a PE idle gap instead
                # of blocking the next step's matmuls in the PE FIFO
                pending_s = (w, pbuf, (slot + 1) * 2 * BL)
                pbuf_prev = pbuf
                pbuf = state.tile([128, WIN * 2 * BL], f16, tag="pb",
                                  name="pb")

    nc.compile()
    return nc


def _prepare(feats, transitions, feats_len):
    f = np.ascontiguousarray(feats.transpose(1, 0, 2)).astype(np.float32)
    # per-step normalizer: mean over batch of logsumexp_k of the frame
    m = f.max(axis=2)
    lse = np.log(np.exp(f - m[:, :, None]).sum(axis=2,
                                               dtype=np.float32)) + m
    c = lse.mean(axis=1).astype(np.float32)             # [T]
    offs = np.cumsum(c.astype(np.float64))              # [T]

    E = np.exp(transitions.astype(np.float32))
    # slot i*KT + k holds E[k-tile, i-tile] (step consumption order)
    e_packed = np.empty((128, KT * KT * 128), np.float16)
    for i in range(KT):
        for k in range(KT):
            e_packed[:, (i * KT + k) * 128:(i * KT + k + 1) * 128] = \
                E[128 * k:128 * (k + 1), 128 * i:128 * (i + 1)]

    # F[t, b, k] = exp(f[t, b, k] - c_t) -> [128, T*2*BL] per core
    Fx = np.exp(f - c[:, None, None]).astype(np.float32)  # [T, B, K]
    f_maps = []
    for core in range(N_CORES):
        sl = Fx[:, core * BL:(core + 1) * BL, :]          # [T, BL, K]
        blk = sl.reshape(T, BL, KT, 128).transpose(3, 0, 2, 1)
        f_maps.append({"f_in": np.ascontiguousarray(
            blk.reshape(128, T * KT * BL)).astype(np.float16)})
    return e_packed, f_maps, offs, c


def _gold_score(feats, transitions, tags, feats_len):
    f = feats.transpose(1, 0, 2).astype(np.float32)       # [T, B, K]
    tg = tags.T.astype(np.int64)                          # [T, B]
    mask = (np.arange(T)[:, None] < feats_len[None, :])
    maskf = mask.astype(np.float32)
    emit = np.take_along_axis(f, tg[:, :, None], axis=2)[:, :, 0] * maskf
    u = emit.sum(axis=0, dtype=np.float32)
    t_mask = maskf[:-1] * maskf[1:]
    t_score = transitions.astype(np.float32)[tg[:-1], tg[1:]] * t_mask
    return (u + t_score.sum(axis=0, dtype=np.float32)).astype(np.float32)


def kernel(feats, transitions, tags, feats_len, _results_hook=None,
           _trace=False):
    from concourse.bass_utils import run_bass_kernel_spmd

    feats = np.asarray(feats, dtype=np.float32)
    transitions = np.asarray(transitions, dtype=np.float32)
    tags_np = np.asarray(tags)
    feats_len_np = np.asarray(feats_len).astype(np.int64)

    # compile only max(len) steps; floor keeps the chunk schedule valid
    t_eff = min(T, max(int(feats_len_np.max()), 2 * CHUNK))
    if ("nc", t_eff) not in _cache:
        _cache[("nc", t_eff)] = _build_nc(t_eff)
    nc = _cache[("nc", t_eff)]

    e_packed, f_maps, offs, _c = _prepare(feats, transitions, feats_len_np)
    in_maps = [{"e_in": e_packed, **f_maps[core]} for core in range(N_CORES)]

    res = run_bass_kernel_spmd(nc, in_maps, core_ids=list(range(N_CORES)),
                               trace=_trace)
    if _results_hook is not None:
        _results_hook(res)

    u = _gold_score(feats, transitions, tags_np, feats_len_np)
    loss = np.empty(B, np.float32)
    idx = feats_len_np - 1                                 # [B] capture step
    for core in range(N_CORES):
        s = res.results[core]["s_out"].reshape(T, KT, BL).astype(
            np.float64).sum(axis=1)                        # [T, BL]
        bl = np.arange(BL)
        bg = core * BL + bl
        sv = s[idx[bg], bl]
        loss[bg] = (np.log(sv) + offs[idx[bg]]).astype(np.float32) - u[bg]
    return loss
